# revision 17
# baseline (speedup 1.0000x reference)
"""Trainium2 Bass kernel for: x + s -> LayerNorm(W) -> 2x2x2 avgpool -> exact GELU.

Input  x: (32, 32, 16, 32, 64) f32, sum_weight (1,), gamma (64,), beta (64,)
Output:   (32, 32, 8, 16, 32) f32

Math notes:
  v = x + s;  LN over last dim W: mean/var are shift-equivariant/invariant, so
  sum_weight cancels exactly.
  ln = (x - mu) * rho * gamma + beta,  rho = rsqrt(var + eps)
  pooled[q, w'] = (1/8) [ S - mq[q]*gw[w'] + 4*(beta_e+beta_o)[w'] ]
    S  = sum_{r in quad} rho_r * (ga*x[r,2w'] + go*x[r,2w'+1])
    mq = sum_{r in quad} (64*mu_r) * rho_r,  gw = (ga + go)/64
  out = Gelu(pooled)

Implementation strategy:
  - Stats: ACT squares x with a parity-DEINTERLEAVED fp16 output layout
    [rows, 2, 32]; DVE then pair-sums at 2x perf mode (all operands 2-byte
    unit-stride) and row-reduces half-size inputs.  r1 via pair-sum (GP stt)
    + half-size DVE reduce.
  - Per-row scale xr = x * rstd on GPSIMD scalar_tensor_tensor (0.6 impl
    efficiency vs 0.42 for plain TT), fp16 deinterleaved output.
  - d-pool / h-pool / gamma-combine / beta all fp16 unit-stride at DVE 2x.
  - Smalls batched per chunk-pair (128 rows); tail batched per half (4
    chunks).  ACT only loads Square, Sqrt, Gelu tables.

Layout: data-parallel over batch N (4 per core x 8 cores). Partition dim =
128 (n, c) pairs; free dim = (d, h, w).  Chunk k = d in {2k, 2k+1}: 64 LN rows
of W=64 per partition.
"""

import numpy as np

import concourse.bacc as bacc
import concourse.bass as bass
import concourse.tile as tile
from concourse import mybir
from concourse.bass_utils import run_bass_kernel_spmd

P = 128
N, C, D, H, W = 32, 32, 16, 32, 64
NCORES = 8
NPER = N // NCORES
EPS = 1e-5
F32 = mybir.dt.float32
F16 = mybir.dt.float16

CHUNK = 2 * H * W          # 4096 elems / partition, 64 rows of 64
NCHUNK = D // 2            # 8
ROWS = 64                  # rows per chunk
ALU = mybir.AluOpType




def _bcast(ap, shape):
    """Broadcast [P, n] AP to shape (P, ..., n) with stride-0 middle dims."""
    while len(ap.shape) < len(shape):
        ap = ap.unsqueeze(1)
    return ap.to_broadcast(shape)


def _kernel_body(ctx, tc: tile.TileContext, out_ap: bass.AP, xs: bass.AP,
                 cons: bass.AP):
    nc = tc.nc

    singles = ctx.enter_context(tc.tile_pool(name="singles", bufs=1))
    xpool = ctx.enter_context(tc.tile_pool(name="xpool", bufs=4))
    sqpool = ctx.enter_context(tc.tile_pool(name="sqpool", bufs=2))
    pspool = ctx.enter_context(tc.tile_pool(name="pspool", bufs=2))
    xrpool = ctx.enter_context(tc.tile_pool(name="xrpool", bufs=2))
    xdpool = ctx.enter_context(tc.tile_pool(name="xdpool", bufs=2))
    smpool = ctx.enter_context(tc.tile_pool(name="smpool", bufs=2))
    tailpool = ctx.enter_context(tc.tile_pool(name="tailpool", bufs=1))

    # --- first chunk DMAs before constants (cuts startup latency) ---
    xsf_early = xs.rearrange("p d h w -> p (d h w)")
    xc_early = []
    for k in range(2):
        xc = xpool.tile([P, CHUNK], F32, tag="xc", name=f"xce{k}")
        nc.sync.dma_start(out=xc[:],
                          in_=xsf_early[:, k * CHUNK:(k + 1) * CHUNK])
        xc_early.append(xc)

    # --- constants ---
    ga_t = singles.tile([P, 32], F32)
    go_t = singles.tile([P, 32], F32)
    gw_t = singles.tile([P, 32], F32)
    bw_t = singles.tile([P, 32], F32)
    for r, t in enumerate((ga_t, go_t, gw_t, bw_t)):
        nc.sync.dma_start(out=t[:], in_=cons[r:r + 1, :].to_broadcast((P, 32)))
    ga16_t = singles.tile([P, 32], F16)
    nc.vector.tensor_scalar_mul(out=ga16_t[:], in0=ga_t[:], scalar1=1.0)
    go16_t = singles.tile([P, 32], F16)
    nc.vector.tensor_scalar_mul(out=go16_t[:], in0=go_t[:], scalar1=1.0)
    bw16_t = singles.tile([P, 32], F16)
    nc.vector.tensor_scalar_mul(out=bw16_t[:], in0=bw_t[:], scalar1=1.0)
    eps_t = singles.tile([P, 1], F32)
    nc.vector.memset(eps_t[:], EPS)

    xsf = xs.rearrange("p d h w -> p (d h w)")
    outf = out_ap.rearrange("p d h w -> p (d h w)")  # [P, 4096]

    # --- persistent staging ---
    # xh layout per half: [P, 4 chunks, 16 h', 2 parity, 32 w'] fp16
    xh_half = [singles.tile([P, 4, 16, 2, 32], F16, name=f"xh{i}")
               for i in range(2)]
    rstd_p = [singles.tile([P, 2 * ROWS], F32, name=f"rstd{i}")
              for i in range(4)]
    r1_p = [singles.tile([P, 2 * ROWS], F32, name=f"r1v{i}")
            for i in range(4)]
    r2_p = [singles.tile([P, 2 * ROWS], F32, name=f"r2v{i}")
            for i in range(4)]
    mr_half = [singles.tile([P, 4 * ROWS], F32, name=f"mr{i}")
               for i in range(2)]

    def dma_in(k):
        xc = xpool.tile([P, CHUNK], F32, tag="xc")
        nc.sync.dma_start(out=xc[:], in_=xsf[:, k * CHUNK:(k + 1) * CHUNK])
        return xc

    def stats(k, xc):
        """ACT square (parity-outer fp16) + DVE psq + row reduces.

        Reduces are SBUF-contention-immune, so they are what co-runs with
        GPSIMD xr ops; the psq TT is small."""
        p, kk = k // 2, k % 2
        # x viewed as [P, parity, row, w'] (parity OUTER -> contiguous halves)
        x4o = xc[:].rearrange("p (r v t) -> p t r v", v=32, t=2)
        sq4 = sqpool.tile([P, 2, ROWS, 32], F16, tag="sq")
        nc.scalar.activation(sq4[:], x4o,
                             mybir.ActivationFunctionType.Square)
        psq = pspool.tile([P, ROWS, 32], F16, tag="psq")
        nc.vector.tensor_tensor(out=psq[:], in0=sq4[:, 0, :, :],
                                in1=sq4[:, 1, :, :], op=ALU.add)
        nc.vector.tensor_reduce(out=r2_p[p][:, kk * ROWS:(kk + 1) * ROWS],
                                in_=psq[:], axis=mybir.AxisListType.X,
                                op=ALU.add)
        x3 = xc[:].rearrange("p (r w) -> p r w", w=W)
        nc.vector.tensor_reduce(out=r1_p[p][:, kk * ROWS:(kk + 1) * ROWS],
                                in_=x3, axis=mybir.AxisListType.X,
                                op=ALU.add)

    def smalls(p):
        """Stats recombination for a pair (128 rows): rstd, mr = 64*mu*rstd."""
        r1v, r2v = r1_p[p][:], r2_p[p][:]
        sqm = smpool.tile([P, 2 * ROWS], F32, tag="sqm")
        nc.vector.tensor_tensor(out=sqm[:], in0=r1v, in1=r1v, op=ALU.mult)
        # v64 = r2 - sqm/64  (= 64 * var)
        v64 = smpool.tile([P, 2 * ROWS], F32, tag="v64")
        nc.vector.scalar_tensor_tensor(out=v64[:], in0=sqm[:],
                                       scalar=-1.0 / W, in1=r2v,
                                       op0=ALU.mult, op1=ALU.add)
        sd = smpool.tile([P, 2 * ROWS], F32, tag="sd")
        nc.scalar.activation(sd[:], v64[:],
                             mybir.ActivationFunctionType.Sqrt,
                             bias=eps_t[:], scale=1.0 / W)
        rt = rstd_p[p]
        nc.vector.reciprocal(out=rt[:], in_=sd[:])
        mrh = mr_half[p // 2]
        nc.vector.tensor_tensor(out=mrh[:, (p % 2) * 128:(p % 2) * 128 + 128],
                                in0=r1v, in1=rt[:], op=ALU.mult)

    def xr_op(k, xc):
        """xr = x*rstd (fp16, deinterleaved out) on GPSIMD."""
        p, kk = k // 2, k % 2
        rt = rstd_p[p][:, kk * ROWS:(kk + 1) * ROWS]  # [P, 64]
        x4 = xc[:].rearrange("p (r v t) -> p r t v", v=32, t=2)
        xr = xrpool.tile([P, ROWS, 2, 32], F16, tag="xr")
        rb = rt.unsqueeze(2).unsqueeze(3).to_broadcast((P, ROWS, 2, 32))
        nc.gpsimd.tensor_tensor(out=xr[:], in0=x4, in1=rb, op=ALU.mult)
        return xr

    def pools(k, xr):
        """d-pool + h-pool into xh_half (DVE fp16 2x)."""
        # d-pool: [P, 2, 2048] -> [P, 2048] (contiguous halves)
        xd = xdpool.tile([P, CHUNK // 2], F16, tag="xd")
        xr2 = xr[:].rearrange("p r t v -> p (r t v)").rearrange(
            "p (s f) -> p s f", s=2)
        nc.vector.tensor_tensor(out=xd[:], in0=xr2[:, 0, :], in1=xr2[:, 1, :],
                                op=ALU.add)
        # h-pool: [P, 16, 2, 64] -> xh_half[:, k%4]; 64 = (t, v)
        xd3 = xd[:].rearrange("p (h s f) -> p h s f", s=2, f=64)
        xho = xh_half[k // 4][:, k % 4, :, :, :].rearrange(
            "p h t v -> p h (t v)")
        nc.vector.tensor_tensor(out=xho, in0=xd3[:, :, 0, :],
                                in1=xd3[:, :, 1, :], op=ALU.add)

    def tail_dve(h):
        """Mean correction + gamma combine + beta + GELU for chunks 4h..4h+3.

        All on DVE/ACT; scheduled in GP-idle windows."""
        mr5 = mr_half[h][:].rearrange("p (k d q t) -> p k d q t", k=4, d=2,
                                      t=2)
        mq1 = tailpool.tile([P, 4, 2, 16], F32, tag="mq1")
        nc.vector.tensor_tensor(out=mq1[:], in0=mr5[:, :, :, :, 0],
                                in1=mr5[:, :, :, :, 1], op=ALU.add)
        mq = tailpool.tile([P, 4, 16], F32, tag="mq")
        nc.vector.tensor_tensor(out=mq[:], in0=mq1[:, :, 0, :],
                                in1=mq1[:, :, 1, :], op=ALU.add)
        sh3 = (P, 64, 32)
        corr = tailpool.tile([P, 64, 32], F16, tag="corr")
        mqb = mq[:].rearrange("p k h -> p (k h)").unsqueeze(2).to_broadcast(
            sh3)
        nc.vector.tensor_tensor(out=corr[:], in0=mqb,
                                in1=_bcast(gw_t[:], sh3), op=ALU.mult)
        xh = xh_half[h][:]  # [P, 4, 16, 2, 32]
        # unit-stride parity slices: [P, (k h), 32]
        xhf = xh.rearrange("p k h t v -> p (k h) t v")
        t1 = tailpool.tile([P, 64, 32], F16, tag="t1")
        nc.vector.tensor_tensor(out=t1[:], in0=xhf[:, :, 0, :],
                                in1=_bcast(ga16_t[:], sh3), op=ALU.mult)
        t2 = tailpool.tile([P, 64, 32], F16, tag="t2")
        nc.vector.tensor_tensor(out=t2[:], in0=xhf[:, :, 1, :],
                                in1=_bcast(go16_t[:], sh3), op=ALU.mult)
        s_t = tailpool.tile([P, 64, 32], F16, tag="s")
        nc.vector.tensor_tensor(out=s_t[:], in0=t1[:], in1=t2[:], op=ALU.add)
        pre = tailpool.tile([P, 64, 32], F16, tag="pre")
        nc.vector.tensor_tensor(out=pre[:], in0=s_t[:], in1=corr[:],
                                op=ALU.subtract)
        pre2 = tailpool.tile([P, 64, 32], F16, tag="pre2")
        nc.vector.tensor_tensor(out=pre2[:], in0=pre[:],
                                in1=_bcast(bw16_t[:], sh3), op=ALU.add)
        res = tailpool.tile([P, 4 * 512], F32, tag="res")
        nc.scalar.activation(res[:], pre2[:].rearrange("p a b -> p (a b)"),
                             mybir.ActivationFunctionType.Gelu, scale=0.125)
        nc.sync.dma_start(out=outf[:, h * 2048:(h + 1) * 2048], in_=res[:])

    # ---- schedule: chunk cadence; GP xr(k) co-runs with DVE reduces of
    # stats(k+2); DVE TT pools run in the GP-idle tail of each slot ----
    xc_t = [None] * NCHUNK
    xc_t[0], xc_t[1] = xc_early
    for k in range(2, 4):
        xc_t[k] = dma_in(k)
    stats(0, xc_t[0])
    stats(1, xc_t[1])
    smalls(0)
    for k in range(NCHUNK):
        xr_k = xr_op(k, xc_t[k])
        if k + 2 < NCHUNK:
            if k + 4 < NCHUNK:
                xc_t[k + 4] = dma_in(k + 4)
            stats(k + 2, xc_t[k + 2])
            if (k + 2) % 2 == 1:
                smalls((k + 2) // 2)
        pools(k, xr_k)
        if k == 3:
            tail_dve(0)
    tail_dve(1)


_CACHE: dict = {}


def _get_compiled():
    if "nc" not in _CACHE:
        nc = bacc.Bacc("TRN2", target_bir_lowering=False, debug=False)
        xs = nc.dram_tensor("xs", [P, D, H, W], F32, kind="ExternalInput").ap()
        cons = nc.dram_tensor("cons", [4, 32], F32, kind="ExternalInput").ap()
        out = nc.dram_tensor(
            "out", [P, D // 2, H // 2, W // 2], F32, kind="ExternalOutput"
        ).ap()
        from contextlib import ExitStack

        with tile.TileContext(nc) as tc, ExitStack() as ctx:
            _kernel_body(ctx, tc, out, xs, cons)
        nc.compile()
        _CACHE["nc"] = nc
    return _CACHE["nc"]


def _make_cons(gamma: np.ndarray, beta: np.ndarray) -> np.ndarray:
    ga = gamma[0::2].astype(np.float64)
    go = gamma[1::2].astype(np.float64)
    # mr carries 64*mu*rstd -> fold the 1/64 into gw
    gw = (ga + go) / 64.0
    bw = 4.0 * (beta[0::2].astype(np.float64) + beta[1::2].astype(np.float64))
    return np.stack([ga, go, gw, bw]).astype(np.float32)


def kernel(x, sum_weight, gamma, beta, trace=False):
    del sum_weight  # cancels exactly in LayerNorm (shift invariance)
    nc = _get_compiled()
    x = np.ascontiguousarray(np.asarray(x), dtype=np.float32)
    cons = _make_cons(np.asarray(gamma), np.asarray(beta))
    in_maps = []
    for core in range(NCORES):
        shard = x[core * NPER:(core + 1) * NPER].reshape(P, D, H, W)
        in_maps.append({"xs": shard, "cons": cons})
    res = run_bass_kernel_spmd(nc, in_maps, core_ids=list(range(NCORES)),
                               trace=trace)
    out = np.concatenate(
        [
            res.results[i]["out"].reshape(NPER, C, D // 2, H // 2, W // 2)
            for i in range(NCORES)
        ],
        axis=0,
    )
    if trace:
        return out, res
    return out


if __name__ == "__main__":
    rng = np.random.default_rng(0)
    x = rng.standard_normal((N, C, D, H, W), dtype=np.float32)
    sw = rng.standard_normal((1,)).astype(np.float32)
    gamma = rng.random((W,), dtype=np.float32)
    beta = rng.standard_normal((W,)).astype(np.float32)
    y = kernel(x, sw, gamma, beta)
    print(y.shape, y.dtype)


# revision 18
# speedup vs baseline: 1.1297x; 1.1297x over previous
"""Trainium2 Bass kernel for: x + s -> LayerNorm(W) -> 2x2x2 avgpool -> exact GELU.

Input  x: (32, 32, 16, 32, 64) f32, sum_weight (1,), gamma (64,), beta (64,)
Output:   (32, 32, 8, 16, 32) f32

Math notes:
  v = x + s;  LN over last dim W: mean/var are shift-equivariant/invariant, so
  sum_weight cancels exactly.
  ln = (x - mu) * rho * gamma + beta,  rho = rsqrt(var + eps)
  pooled[q, w'] = (1/8) [ S - mq[q]*gw[w'] + 4*(beta_e+beta_o)[w'] ]
    S  = sum_{r in quad} rho_r * (ga*x[r,2w'] + go*x[r,2w'+1])
    mq = sum_{r in quad} (64*mu_r) * rho_r,  gw = (ga + go)/64
  out = Gelu(pooled)

Implementation strategy:
  - Stats: ACT squares x with a parity-DEINTERLEAVED fp16 output layout
    [rows, 2, 32]; DVE then pair-sums at 2x perf mode (all operands 2-byte
    unit-stride) and row-reduces half-size inputs.  r1 via pair-sum (GP stt)
    + half-size DVE reduce.
  - Per-row scale xr = x * rstd on GPSIMD scalar_tensor_tensor (0.6 impl
    efficiency vs 0.42 for plain TT), fp16 deinterleaved output.
  - d-pool / h-pool / gamma-combine / beta all fp16 unit-stride at DVE 2x.
  - Smalls batched per chunk-pair (128 rows); tail batched per half (4
    chunks).  ACT only loads Square, Sqrt, Gelu tables.

Layout: data-parallel over batch N (4 per core x 8 cores). Partition dim =
128 (n, c) pairs; free dim = (d, h, w).  Chunk k = d in {2k, 2k+1}: 64 LN rows
of W=64 per partition.
"""

import numpy as np

import concourse.bacc as bacc
import concourse.bass as bass
import concourse.tile as tile
from concourse import mybir
from concourse.bass_utils import run_bass_kernel_spmd

P = 128
N, C, D, H, W = 32, 32, 16, 32, 64
NCORES = 8
NPER = N // NCORES
EPS = 1e-5
F32 = mybir.dt.float32
F16 = mybir.dt.float16

CHUNK = 2 * H * W          # 4096 elems / partition, 64 rows of 64
NCHUNK = D // 2            # 8
ROWS = 64                  # rows per chunk
ALU = mybir.AluOpType




def _bcast(ap, shape):
    """Broadcast [P, n] AP to shape (P, ..., n) with stride-0 middle dims."""
    while len(ap.shape) < len(shape):
        ap = ap.unsqueeze(1)
    return ap.to_broadcast(shape)


def _kernel_body(ctx, tc: tile.TileContext, out_ap: bass.AP, xs: bass.AP,
                 cons: bass.AP):
    nc = tc.nc

    singles = ctx.enter_context(tc.tile_pool(name="singles", bufs=1))
    xpool = ctx.enter_context(tc.tile_pool(name="xpool", bufs=4))
    sqpool = ctx.enter_context(tc.tile_pool(name="sqpool", bufs=2))
    pspool = ctx.enter_context(tc.tile_pool(name="pspool", bufs=2))
    xrpool = ctx.enter_context(tc.tile_pool(name="xrpool", bufs=2))
    xdpool = ctx.enter_context(tc.tile_pool(name="xdpool", bufs=2))
    smpool = ctx.enter_context(tc.tile_pool(name="smpool", bufs=2))
    tailpool = ctx.enter_context(tc.tile_pool(name="tailpool", bufs=1))

    # --- first chunk DMAs before constants (cuts startup latency) ---
    xsf_early = xs.rearrange("p d h w -> p (d h w)")
    xc_early = []
    for k in range(2):
        xc = xpool.tile([P, CHUNK], F32, tag="xc", name=f"xce{k}")
        nc.sync.dma_start(out=xc[:],
                          in_=xsf_early[:, k * CHUNK:(k + 1) * CHUNK])
        xc_early.append(xc)

    # --- constants ---
    ga_t = singles.tile([P, 32], F32)
    go_t = singles.tile([P, 32], F32)
    gw_t = singles.tile([P, 32], F32)
    bw_t = singles.tile([P, 32], F32)
    for r, t in enumerate((ga_t, go_t, gw_t, bw_t)):
        nc.sync.dma_start(out=t[:], in_=cons[r:r + 1, :].to_broadcast((P, 32)))
    ga16_t = singles.tile([P, 32], F16)
    nc.vector.tensor_scalar_mul(out=ga16_t[:], in0=ga_t[:], scalar1=1.0)
    go16_t = singles.tile([P, 32], F16)
    nc.vector.tensor_scalar_mul(out=go16_t[:], in0=go_t[:], scalar1=1.0)
    bw16_t = singles.tile([P, 32], F16)
    nc.vector.tensor_scalar_mul(out=bw16_t[:], in0=bw_t[:], scalar1=1.0)
    eps_t = singles.tile([P, 1], F32)
    nc.vector.memset(eps_t[:], EPS)

    xsf = xs.rearrange("p d h w -> p (d h w)")
    outf = out_ap.rearrange("p d h w -> p (d h w)")  # [P, 4096]

    # --- persistent staging ---
    # xh layout per half: [P, 4 chunks, 16 h', 2 parity, 32 w'] fp16
    xh_half = [singles.tile([P, 4, 16, 2, 32], F16, name=f"xh{i}")
               for i in range(2)]
    rstd_p = [singles.tile([P, 2 * ROWS], F32, name=f"rstd{i}")
              for i in range(4)]
    r1_p = [singles.tile([P, 2 * ROWS], F32, name=f"r1v{i}")
            for i in range(4)]
    r2_p = [singles.tile([P, 2 * ROWS], F32, name=f"r2v{i}")
            for i in range(4)]
    mr_half = [singles.tile([P, 4 * ROWS], F32, name=f"mr{i}")
               for i in range(2)]

    def dma_in(k):
        xc = xpool.tile([P, CHUNK], F32, tag="xc")
        nc.sync.dma_start(out=xc[:], in_=xsf[:, k * CHUNK:(k + 1) * CHUNK])
        return xc

    def stats(k, xc):
        """ACT square (parity-outer fp16) + DVE psq + row reduces.

        Reduces are SBUF-contention-immune, so they are what co-runs with
        GPSIMD xr ops; the psq TT is small."""
        p, kk = k // 2, k % 2
        # x viewed as [P, parity, row, w'] (parity OUTER -> contiguous halves)
        x4o = xc[:].rearrange("p (r v t) -> p t r v", v=32, t=2)
        sq4 = sqpool.tile([P, 2, ROWS, 32], F16, tag="sq")
        nc.scalar.activation(sq4[:], x4o,
                             mybir.ActivationFunctionType.Square)
        psq = pspool.tile([P, ROWS, 32], F16, tag="psq")
        nc.vector.tensor_tensor(out=psq[:], in0=sq4[:, 0, :, :],
                                in1=sq4[:, 1, :, :], op=ALU.add)
        nc.vector.tensor_reduce(out=r2_p[p][:, kk * ROWS:(kk + 1) * ROWS],
                                in_=psq[:], axis=mybir.AxisListType.X,
                                op=ALU.add)
        x3 = xc[:].rearrange("p (r w) -> p r w", w=W)
        nc.vector.tensor_reduce(out=r1_p[p][:, kk * ROWS:(kk + 1) * ROWS],
                                in_=x3, axis=mybir.AxisListType.X,
                                op=ALU.add)

    def smalls(p):
        """Stats recombination for a pair (128 rows): rstd, mr = 64*mu*rstd."""
        r1v, r2v = r1_p[p][:], r2_p[p][:]
        sqm = smpool.tile([P, 2 * ROWS], F32, tag="sqm")
        nc.vector.tensor_tensor(out=sqm[:], in0=r1v, in1=r1v, op=ALU.mult)
        # v64 = r2 - sqm/64  (= 64 * var)
        v64 = smpool.tile([P, 2 * ROWS], F32, tag="v64")
        nc.vector.scalar_tensor_tensor(out=v64[:], in0=sqm[:],
                                       scalar=-1.0 / W, in1=r2v,
                                       op0=ALU.mult, op1=ALU.add)
        sd = smpool.tile([P, 2 * ROWS], F32, tag="sd")
        nc.scalar.activation(sd[:], v64[:],
                             mybir.ActivationFunctionType.Sqrt,
                             bias=eps_t[:], scale=1.0 / W)
        rt = rstd_p[p]
        nc.vector.reciprocal(out=rt[:], in_=sd[:])
        mrh = mr_half[p // 2]
        nc.vector.tensor_tensor(out=mrh[:, (p % 2) * 128:(p % 2) * 128 + 128],
                                in0=r1v, in1=rt[:], op=ALU.mult)

    def xr_op(k, xc):
        """xr = x*rstd (fp16, deinterleaved out) on DVE (broadcast reads are
        full speed when GPSIMD is idle)."""
        p, kk = k // 2, k % 2
        rt = rstd_p[p][:, kk * ROWS:(kk + 1) * ROWS]  # [P, 64]
        x4 = xc[:].rearrange("p (r v t) -> p r t v", v=32, t=2)
        xr = xrpool.tile([P, ROWS, 2, 32], F16, tag="xr")
        rb = rt.unsqueeze(2).unsqueeze(3).to_broadcast((P, ROWS, 2, 32))
        nc.vector.tensor_tensor(out=xr[:], in0=x4, in1=rb, op=ALU.mult)
        return xr

    def pools(k, xr):
        """d-pool + h-pool into xh_half (DVE fp16 2x)."""
        # d-pool: [P, 2, 2048] -> [P, 2048] (contiguous halves)
        xd = xdpool.tile([P, CHUNK // 2], F16, tag="xd")
        xr2 = xr[:].rearrange("p r t v -> p (r t v)").rearrange(
            "p (s f) -> p s f", s=2)
        nc.vector.tensor_tensor(out=xd[:], in0=xr2[:, 0, :], in1=xr2[:, 1, :],
                                op=ALU.add)
        # h-pool: [P, 16, 2, 64] -> xh_half[:, k%4]; 64 = (t, v)
        xd3 = xd[:].rearrange("p (h s f) -> p h s f", s=2, f=64)
        xho = xh_half[k // 4][:, k % 4, :, :, :].rearrange(
            "p h t v -> p h (t v)")
        nc.vector.tensor_tensor(out=xho, in0=xd3[:, :, 0, :],
                                in1=xd3[:, :, 1, :], op=ALU.add)

    def tail_dve(h):
        """Mean correction + gamma combine + beta + GELU for chunks 4h..4h+3.

        All on DVE/ACT; scheduled in GP-idle windows."""
        mr5 = mr_half[h][:].rearrange("p (k d q t) -> p k d q t", k=4, d=2,
                                      t=2)
        mq1 = tailpool.tile([P, 4, 2, 16], F32, tag="mq1")
        nc.vector.tensor_tensor(out=mq1[:], in0=mr5[:, :, :, :, 0],
                                in1=mr5[:, :, :, :, 1], op=ALU.add)
        mq = tailpool.tile([P, 4, 16], F32, tag="mq")
        nc.vector.tensor_tensor(out=mq[:], in0=mq1[:, :, 0, :],
                                in1=mq1[:, :, 1, :], op=ALU.add)
        sh3 = (P, 64, 32)
        corr = tailpool.tile([P, 64, 32], F16, tag="corr")
        mqb = mq[:].rearrange("p k h -> p (k h)").unsqueeze(2).to_broadcast(
            sh3)
        nc.vector.tensor_tensor(out=corr[:], in0=mqb,
                                in1=_bcast(gw_t[:], sh3), op=ALU.mult)
        xh = xh_half[h][:]  # [P, 4, 16, 2, 32]
        # unit-stride parity slices: [P, (k h), 32]
        xhf = xh.rearrange("p k h t v -> p (k h) t v")
        t1 = tailpool.tile([P, 64, 32], F16, tag="t1")
        nc.vector.tensor_tensor(out=t1[:], in0=xhf[:, :, 0, :],
                                in1=_bcast(ga16_t[:], sh3), op=ALU.mult)
        t2 = tailpool.tile([P, 64, 32], F16, tag="t2")
        nc.vector.tensor_tensor(out=t2[:], in0=xhf[:, :, 1, :],
                                in1=_bcast(go16_t[:], sh3), op=ALU.mult)
        s_t = tailpool.tile([P, 64, 32], F16, tag="s")
        nc.vector.tensor_tensor(out=s_t[:], in0=t1[:], in1=t2[:], op=ALU.add)
        pre = tailpool.tile([P, 64, 32], F16, tag="pre")
        nc.vector.tensor_tensor(out=pre[:], in0=s_t[:], in1=corr[:],
                                op=ALU.subtract)
        pre2 = tailpool.tile([P, 64, 32], F16, tag="pre2")
        nc.vector.tensor_tensor(out=pre2[:], in0=pre[:],
                                in1=_bcast(bw16_t[:], sh3), op=ALU.add)
        res = tailpool.tile([P, 4 * 512], F32, tag="res")
        nc.scalar.activation(res[:], pre2[:].rearrange("p a b -> p (a b)"),
                             mybir.ActivationFunctionType.Gelu, scale=0.125)
        nc.sync.dma_start(out=outf[:, h * 2048:(h + 1) * 2048], in_=res[:])

    # ---- schedule: chunk cadence; GP xr(k) co-runs with DVE reduces of
    # stats(k+2); DVE TT pools run in the GP-idle tail of each slot ----
    xc_t = [None] * NCHUNK
    xc_t[0], xc_t[1] = xc_early
    for k in range(2, 4):
        xc_t[k] = dma_in(k)
    stats(0, xc_t[0])
    stats(1, xc_t[1])
    smalls(0)
    for k in range(NCHUNK):
        xr_k = xr_op(k, xc_t[k])
        if k + 2 < NCHUNK:
            if k + 4 < NCHUNK:
                xc_t[k + 4] = dma_in(k + 4)
            stats(k + 2, xc_t[k + 2])
            if (k + 2) % 2 == 1:
                smalls((k + 2) // 2)
        pools(k, xr_k)
        if k == 3:
            tail_dve(0)
    tail_dve(1)


_CACHE: dict = {}


def _get_compiled():
    if "nc" not in _CACHE:
        nc = bacc.Bacc("TRN2", target_bir_lowering=False, debug=False)
        xs = nc.dram_tensor("xs", [P, D, H, W], F32, kind="ExternalInput").ap()
        cons = nc.dram_tensor("cons", [4, 32], F32, kind="ExternalInput").ap()
        out = nc.dram_tensor(
            "out", [P, D // 2, H // 2, W // 2], F32, kind="ExternalOutput"
        ).ap()
        from contextlib import ExitStack

        with tile.TileContext(nc) as tc, ExitStack() as ctx:
            _kernel_body(ctx, tc, out, xs, cons)
        nc.compile()
        _CACHE["nc"] = nc
    return _CACHE["nc"]


def _make_cons(gamma: np.ndarray, beta: np.ndarray) -> np.ndarray:
    ga = gamma[0::2].astype(np.float64)
    go = gamma[1::2].astype(np.float64)
    # mr carries 64*mu*rstd -> fold the 1/64 into gw
    gw = (ga + go) / 64.0
    bw = 4.0 * (beta[0::2].astype(np.float64) + beta[1::2].astype(np.float64))
    return np.stack([ga, go, gw, bw]).astype(np.float32)


def kernel(x, sum_weight, gamma, beta, trace=False):
    del sum_weight  # cancels exactly in LayerNorm (shift invariance)
    nc = _get_compiled()
    x = np.ascontiguousarray(np.asarray(x), dtype=np.float32)
    cons = _make_cons(np.asarray(gamma), np.asarray(beta))
    in_maps = []
    for core in range(NCORES):
        shard = x[core * NPER:(core + 1) * NPER].reshape(P, D, H, W)
        in_maps.append({"xs": shard, "cons": cons})
    res = run_bass_kernel_spmd(nc, in_maps, core_ids=list(range(NCORES)),
                               trace=trace)
    out = np.concatenate(
        [
            res.results[i]["out"].reshape(NPER, C, D // 2, H // 2, W // 2)
            for i in range(NCORES)
        ],
        axis=0,
    )
    if trace:
        return out, res
    return out


if __name__ == "__main__":
    rng = np.random.default_rng(0)
    x = rng.standard_normal((N, C, D, H, W), dtype=np.float32)
    sw = rng.standard_normal((1,)).astype(np.float32)
    gamma = rng.random((W,), dtype=np.float32)
    beta = rng.standard_normal((W,)).astype(np.float32)
    y = kernel(x, sw, gamma, beta)
    print(y.shape, y.dtype)


# revision 19
# speedup vs baseline: 1.1689x; 1.0347x over previous
"""Trainium2 Bass kernel for: x + s -> LayerNorm(W) -> 2x2x2 avgpool -> exact GELU.

Input  x: (32, 32, 16, 32, 64) f32, sum_weight (1,), gamma (64,), beta (64,)
Output:   (32, 32, 8, 16, 32) f32

Math notes:
  v = x + s;  LN over last dim W: mean/var are shift-equivariant/invariant, so
  sum_weight cancels exactly.
  ln = (x - mu) * rho * gamma + beta,  rho = rsqrt(var + eps)
  pooled[q, w'] = (1/8) [ S - mq[q]*gw[w'] + 4*(beta_e+beta_o)[w'] ]
    S  = sum_{r in quad} rho_r * (ga*x[r,2w'] + go*x[r,2w'+1])
    mq = sum_{r in quad} (64*mu_r) * rho_r,  gw = (ga + go)/64
  out = Gelu(pooled)

Implementation strategy:
  - Stats: ACT squares x with a parity-DEINTERLEAVED fp16 output layout
    [rows, 2, 32]; DVE then pair-sums at 2x perf mode (all operands 2-byte
    unit-stride) and row-reduces half-size inputs.  r1 via pair-sum (GP stt)
    + half-size DVE reduce.
  - Per-row scale xr = x * rstd on GPSIMD scalar_tensor_tensor (0.6 impl
    efficiency vs 0.42 for plain TT), fp16 deinterleaved output.
  - d-pool / h-pool / gamma-combine / beta all fp16 unit-stride at DVE 2x.
  - Smalls batched per chunk-pair (128 rows); tail batched per half (4
    chunks).  ACT only loads Square, Sqrt, Gelu tables.

Layout: data-parallel over batch N (4 per core x 8 cores). Partition dim =
128 (n, c) pairs; free dim = (d, h, w).  Chunk k = d in {2k, 2k+1}: 64 LN rows
of W=64 per partition.
"""

import numpy as np

import concourse.bacc as bacc
import concourse.bass as bass
import concourse.tile as tile
from concourse import mybir
from concourse.bass_utils import run_bass_kernel_spmd

P = 128
N, C, D, H, W = 32, 32, 16, 32, 64
NCORES = 8
NPER = N // NCORES
EPS = 1e-5
F32 = mybir.dt.float32
F16 = mybir.dt.float16

CHUNK = 2 * H * W          # 4096 elems / partition, 64 rows of 64
NCHUNK = D // 2            # 8
ROWS = 64                  # rows per chunk
ALU = mybir.AluOpType




def _bcast(ap, shape):
    """Broadcast [P, n] AP to shape (P, ..., n) with stride-0 middle dims."""
    while len(ap.shape) < len(shape):
        ap = ap.unsqueeze(1)
    return ap.to_broadcast(shape)


def _kernel_body(ctx, tc: tile.TileContext, out_ap: bass.AP, xs: bass.AP,
                 cons: bass.AP):
    nc = tc.nc

    singles = ctx.enter_context(tc.tile_pool(name="singles", bufs=1))
    xpool = ctx.enter_context(tc.tile_pool(name="xpool", bufs=4))
    sqpool = ctx.enter_context(tc.tile_pool(name="sqpool", bufs=2))
    pspool = ctx.enter_context(tc.tile_pool(name="pspool", bufs=2))
    xrpool = ctx.enter_context(tc.tile_pool(name="xrpool", bufs=2))
    xdpool = ctx.enter_context(tc.tile_pool(name="xdpool", bufs=2))
    smpool = ctx.enter_context(tc.tile_pool(name="smpool", bufs=2))
    tailpool = ctx.enter_context(tc.tile_pool(name="tailpool", bufs=2))

    # --- first chunk DMAs before constants (cuts startup latency) ---
    xsf_early = xs.rearrange("p d h w -> p (d h w)")
    xc_early = []
    for k in range(2):
        xc = xpool.tile([P, CHUNK], F32, tag="xc", name=f"xce{k}")
        nc.sync.dma_start(out=xc[:],
                          in_=xsf_early[:, k * CHUNK:(k + 1) * CHUNK])
        xc_early.append(xc)

    # --- constants ---
    ga_t = singles.tile([P, 32], F32)
    go_t = singles.tile([P, 32], F32)
    gw_t = singles.tile([P, 32], F32)
    bw_t = singles.tile([P, 32], F32)
    for r, t in enumerate((ga_t, go_t, gw_t, bw_t)):
        nc.sync.dma_start(out=t[:], in_=cons[r:r + 1, :].to_broadcast((P, 32)))
    ga16_t = singles.tile([P, 32], F16)
    nc.vector.tensor_scalar_mul(out=ga16_t[:], in0=ga_t[:], scalar1=1.0)
    go16_t = singles.tile([P, 32], F16)
    nc.vector.tensor_scalar_mul(out=go16_t[:], in0=go_t[:], scalar1=1.0)
    bw16_t = singles.tile([P, 32], F16)
    nc.vector.tensor_scalar_mul(out=bw16_t[:], in0=bw_t[:], scalar1=1.0)
    eps_t = singles.tile([P, 1], F32)
    nc.vector.memset(eps_t[:], EPS)

    xsf = xs.rearrange("p d h w -> p (d h w)")
    outf = out_ap.rearrange("p d h w -> p (d h w)")  # [P, 4096]

    # --- persistent staging ---
    # xh layout per pair: [P, 2 chunks, 16 h', 2 parity, 32 w'] fp16
    xh_pair = [singles.tile([P, 2, 16, 2, 32], F16, name=f"xh{i}")
               for i in range(4)]
    rstd_p = [singles.tile([P, 2 * ROWS], F32, name=f"rstd{i}")
              for i in range(4)]
    r1_p = [singles.tile([P, 2 * ROWS], F32, name=f"r1v{i}")
            for i in range(4)]
    r2_p = [singles.tile([P, 2 * ROWS], F32, name=f"r2v{i}")
            for i in range(4)]
    mr_half = [singles.tile([P, 4 * ROWS], F32, name=f"mr{i}")
               for i in range(2)]

    def dma_in(k):
        xc = xpool.tile([P, CHUNK], F32, tag="xc")
        nc.sync.dma_start(out=xc[:], in_=xsf[:, k * CHUNK:(k + 1) * CHUNK])
        return xc

    def stats(k, xc):
        """ACT square (parity-outer fp16) + DVE psq + row reduces.

        Reduces are SBUF-contention-immune, so they are what co-runs with
        GPSIMD xr ops; the psq TT is small."""
        p, kk = k // 2, k % 2
        # x viewed as [P, parity, row, w'] (parity OUTER -> contiguous halves)
        x4o = xc[:].rearrange("p (r v t) -> p t r v", v=32, t=2)
        sq4 = sqpool.tile([P, 2, ROWS, 32], F16, tag="sq")
        nc.scalar.activation(sq4[:], x4o,
                             mybir.ActivationFunctionType.Square)
        psq = pspool.tile([P, ROWS, 32], F16, tag="psq")
        nc.vector.tensor_tensor(out=psq[:], in0=sq4[:, 0, :, :],
                                in1=sq4[:, 1, :, :], op=ALU.add)
        nc.vector.tensor_reduce(out=r2_p[p][:, kk * ROWS:(kk + 1) * ROWS],
                                in_=psq[:], axis=mybir.AxisListType.X,
                                op=ALU.add)
        x4 = xc[:].rearrange("p (r v t) -> p r t v", v=32, t=2)
        ps = pspool.tile([P, ROWS, 32], F32, tag="ps")
        nc.gpsimd.tensor_tensor(out=ps[:], in0=x4[:, :, 0, :],
                                in1=x4[:, :, 1, :], op=ALU.add)
        nc.vector.tensor_reduce(out=r1_p[p][:, kk * ROWS:(kk + 1) * ROWS],
                                in_=ps[:], axis=mybir.AxisListType.X,
                                op=ALU.add)

    def smalls(p):
        """Stats recombination for a pair (128 rows): rstd, mr = 64*mu*rstd."""
        r1v, r2v = r1_p[p][:], r2_p[p][:]
        sqm = smpool.tile([P, 2 * ROWS], F32, tag="sqm")
        nc.vector.tensor_tensor(out=sqm[:], in0=r1v, in1=r1v, op=ALU.mult)
        # v64 = r2 - sqm/64  (= 64 * var)
        v64 = smpool.tile([P, 2 * ROWS], F32, tag="v64")
        nc.vector.scalar_tensor_tensor(out=v64[:], in0=sqm[:],
                                       scalar=-1.0 / W, in1=r2v,
                                       op0=ALU.mult, op1=ALU.add)
        sd = smpool.tile([P, 2 * ROWS], F32, tag="sd")
        nc.scalar.activation(sd[:], v64[:],
                             mybir.ActivationFunctionType.Sqrt,
                             bias=eps_t[:], scale=1.0 / W)
        rt = rstd_p[p]
        nc.vector.reciprocal(out=rt[:], in_=sd[:])
        mrh = mr_half[p // 2]
        nc.vector.tensor_tensor(out=mrh[:, (p % 2) * 128:(p % 2) * 128 + 128],
                                in0=r1v, in1=rt[:], op=ALU.mult)

    def xr_op(k, xc):
        """xr = x*rstd (fp16, deinterleaved out) on DVE (broadcast reads are
        full speed when GPSIMD is idle)."""
        p, kk = k // 2, k % 2
        rt = rstd_p[p][:, kk * ROWS:(kk + 1) * ROWS]  # [P, 64]
        x4 = xc[:].rearrange("p (r v t) -> p r t v", v=32, t=2)
        xr = xrpool.tile([P, ROWS, 2, 32], F16, tag="xr")
        rb = rt.unsqueeze(2).unsqueeze(3).to_broadcast((P, ROWS, 2, 32))
        nc.vector.tensor_tensor(out=xr[:], in0=x4, in1=rb, op=ALU.mult)
        return xr

    def pools(k, xr):
        """d-pool + h-pool into xh_half (DVE fp16 2x)."""
        # d-pool: [P, 2, 2048] -> [P, 2048] (contiguous halves)
        xd = xdpool.tile([P, CHUNK // 2], F16, tag="xd")
        xr2 = xr[:].rearrange("p r t v -> p (r t v)").rearrange(
            "p (s f) -> p s f", s=2)
        nc.vector.tensor_tensor(out=xd[:], in0=xr2[:, 0, :], in1=xr2[:, 1, :],
                                op=ALU.add)
        # h-pool: [P, 16, 2, 64] -> xh_half[:, k%4]; 64 = (t, v)
        xd3 = xd[:].rearrange("p (h s f) -> p h s f", s=2, f=64)
        xho = xh_pair[k // 2][:, k % 2, :, :, :].rearrange(
            "p h t v -> p h (t v)")
        nc.vector.tensor_tensor(out=xho, in0=xd3[:, :, 0, :],
                                in1=xd3[:, :, 1, :], op=ALU.add)

    def tail_dve(p):
        """Mean correction + gamma combine + beta + GELU for pair p."""
        mr5 = mr_half[p // 2][:, (p % 2) * 128:(p % 2) * 128 + 128].rearrange(
            "p (k d q t) -> p k d q t", k=2, d=2, t=2)
        mq1 = tailpool.tile([P, 2, 2, 16], F32, tag="mq1")
        nc.vector.tensor_tensor(out=mq1[:], in0=mr5[:, :, :, :, 0],
                                in1=mr5[:, :, :, :, 1], op=ALU.add)
        mq = tailpool.tile([P, 2, 16], F32, tag="mq")
        nc.vector.tensor_tensor(out=mq[:], in0=mq1[:, :, 0, :],
                                in1=mq1[:, :, 1, :], op=ALU.add)
        sh3 = (P, 32, 32)
        corr = tailpool.tile([P, 32, 32], F16, tag="corr")
        mqb = mq[:].rearrange("p k h -> p (k h)").unsqueeze(2).to_broadcast(
            sh3)
        nc.vector.tensor_tensor(out=corr[:], in0=mqb,
                                in1=_bcast(gw_t[:], sh3), op=ALU.mult)
        xh = xh_pair[p][:]  # [P, 2, 16, 2, 32]
        xhf = xh.rearrange("p k h t v -> p (k h) t v")
        t1 = tailpool.tile([P, 32, 32], F16, tag="t1")
        nc.vector.tensor_tensor(out=t1[:], in0=xhf[:, :, 0, :],
                                in1=_bcast(ga16_t[:], sh3), op=ALU.mult)
        t2 = tailpool.tile([P, 32, 32], F16, tag="t2")
        nc.vector.tensor_tensor(out=t2[:], in0=xhf[:, :, 1, :],
                                in1=_bcast(go16_t[:], sh3), op=ALU.mult)
        s_t = tailpool.tile([P, 32, 32], F16, tag="s")
        nc.vector.tensor_tensor(out=s_t[:], in0=t1[:], in1=t2[:], op=ALU.add)
        pre = tailpool.tile([P, 32, 32], F16, tag="pre")
        nc.vector.tensor_tensor(out=pre[:], in0=s_t[:], in1=corr[:],
                                op=ALU.subtract)
        pre2 = tailpool.tile([P, 32, 32], F16, tag="pre2")
        nc.vector.tensor_tensor(out=pre2[:], in0=pre[:],
                                in1=_bcast(bw16_t[:], sh3), op=ALU.add)
        res = tailpool.tile([P, 2 * 512], F32, tag="res")
        nc.scalar.activation(res[:], pre2[:].rearrange("p a b -> p (a b)"),
                             mybir.ActivationFunctionType.Gelu, scale=0.125)
        nc.sync.dma_start(out=outf[:, p * 1024:(p + 1) * 1024], in_=res[:])

    # ---- schedule: chunk cadence; GP xr(k) co-runs with DVE reduces of
    # stats(k+2); DVE TT pools run in the GP-idle tail of each slot ----
    xc_t = [None] * NCHUNK
    xc_t[0], xc_t[1] = xc_early
    for k in range(2, 4):
        xc_t[k] = dma_in(k)
    stats(0, xc_t[0])
    stats(1, xc_t[1])
    smalls(0)
    for k in range(NCHUNK):
        xr_k = xr_op(k, xc_t[k])
        if k + 2 < NCHUNK:
            if k + 4 < NCHUNK:
                xc_t[k + 4] = dma_in(k + 4)
            stats(k + 2, xc_t[k + 2])
            if (k + 2) % 2 == 1:
                smalls((k + 2) // 2)
        pools(k, xr_k)
        if k >= 1 and k % 2 == 1:
            tail_dve(k // 2)


_CACHE: dict = {}


def _get_compiled():
    if "nc" not in _CACHE:
        nc = bacc.Bacc("TRN2", target_bir_lowering=False, debug=False)
        xs = nc.dram_tensor("xs", [P, D, H, W], F32, kind="ExternalInput").ap()
        cons = nc.dram_tensor("cons", [4, 32], F32, kind="ExternalInput").ap()
        out = nc.dram_tensor(
            "out", [P, D // 2, H // 2, W // 2], F32, kind="ExternalOutput"
        ).ap()
        from contextlib import ExitStack

        with tile.TileContext(nc) as tc, ExitStack() as ctx:
            _kernel_body(ctx, tc, out, xs, cons)
        nc.compile()
        _CACHE["nc"] = nc
    return _CACHE["nc"]


def _make_cons(gamma: np.ndarray, beta: np.ndarray) -> np.ndarray:
    ga = gamma[0::2].astype(np.float64)
    go = gamma[1::2].astype(np.float64)
    # mr carries 64*mu*rstd -> fold the 1/64 into gw
    gw = (ga + go) / 64.0
    bw = 4.0 * (beta[0::2].astype(np.float64) + beta[1::2].astype(np.float64))
    return np.stack([ga, go, gw, bw]).astype(np.float32)


def kernel(x, sum_weight, gamma, beta, trace=False):
    del sum_weight  # cancels exactly in LayerNorm (shift invariance)
    nc = _get_compiled()
    x = np.ascontiguousarray(np.asarray(x), dtype=np.float32)
    cons = _make_cons(np.asarray(gamma), np.asarray(beta))
    in_maps = []
    for core in range(NCORES):
        shard = x[core * NPER:(core + 1) * NPER].reshape(P, D, H, W)
        in_maps.append({"xs": shard, "cons": cons})
    res = run_bass_kernel_spmd(nc, in_maps, core_ids=list(range(NCORES)),
                               trace=trace)
    out = np.concatenate(
        [
            res.results[i]["out"].reshape(NPER, C, D // 2, H // 2, W // 2)
            for i in range(NCORES)
        ],
        axis=0,
    )
    if trace:
        return out, res
    return out


if __name__ == "__main__":
    rng = np.random.default_rng(0)
    x = rng.standard_normal((N, C, D, H, W), dtype=np.float32)
    sw = rng.standard_normal((1,)).astype(np.float32)
    gamma = rng.random((W,), dtype=np.float32)
    beta = rng.standard_normal((W,)).astype(np.float32)
    y = kernel(x, sw, gamma, beta)
    print(y.shape, y.dtype)


# revision 21
# speedup vs baseline: 1.1704x; 1.0013x over previous
"""Trainium2 Bass kernel for: x + s -> LayerNorm(W) -> 2x2x2 avgpool -> exact GELU.

Input  x: (32, 32, 16, 32, 64) f32, sum_weight (1,), gamma (64,), beta (64,)
Output:   (32, 32, 8, 16, 32) f32

Math notes:
  v = x + s;  LN over last dim W: mean/var are shift-equivariant/invariant, so
  sum_weight cancels exactly.
  ln = (x - mu) * rho * gamma + beta,  rho = rsqrt(var + eps)
  pooled[q, w'] = (1/8) [ S - mq[q]*gw[w'] + 4*(beta_e+beta_o)[w'] ]
    S  = sum_{r in quad} rho_r * (ga*x[r,2w'] + go*x[r,2w'+1])
    mq = sum_{r in quad} (64*mu_r) * rho_r,  gw = (ga + go)/64
  out = Gelu(pooled)

Implementation strategy:
  - Stats: ACT squares x with a parity-DEINTERLEAVED fp16 output layout
    [rows, 2, 32]; DVE then pair-sums at 2x perf mode (all operands 2-byte
    unit-stride) and row-reduces half-size inputs.  r1 via pair-sum (GP stt)
    + half-size DVE reduce.
  - Per-row scale xr = x * rstd on GPSIMD scalar_tensor_tensor (0.6 impl
    efficiency vs 0.42 for plain TT), fp16 deinterleaved output.
  - d-pool / h-pool / gamma-combine / beta all fp16 unit-stride at DVE 2x.
  - Smalls batched per chunk-pair (128 rows); tail batched per half (4
    chunks).  ACT only loads Square, Sqrt, Gelu tables.

Layout: data-parallel over batch N (4 per core x 8 cores). Partition dim =
128 (n, c) pairs; free dim = (d, h, w).  Chunk k = d in {2k, 2k+1}: 64 LN rows
of W=64 per partition.
"""

import numpy as np

import concourse.bacc as bacc
import concourse.bass as bass
import concourse.tile as tile
from concourse import mybir
from concourse.bass_utils import run_bass_kernel_spmd

P = 128
N, C, D, H, W = 32, 32, 16, 32, 64
NCORES = 8
NPER = N // NCORES
EPS = 1e-5
F32 = mybir.dt.float32
F16 = mybir.dt.float16

CHUNK = 2 * H * W          # 4096 elems / partition, 64 rows of 64
NCHUNK = D // 2            # 8
ROWS = 64                  # rows per chunk
ALU = mybir.AluOpType




def _bcast(ap, shape):
    """Broadcast [P, n] AP to shape (P, ..., n) with stride-0 middle dims."""
    while len(ap.shape) < len(shape):
        ap = ap.unsqueeze(1)
    return ap.to_broadcast(shape)


def _kernel_body(ctx, tc: tile.TileContext, out_ap: bass.AP, xs: bass.AP,
                 cons: bass.AP):
    nc = tc.nc

    singles = ctx.enter_context(tc.tile_pool(name="singles", bufs=1))
    xpool = ctx.enter_context(tc.tile_pool(name="xpool", bufs=4))
    sqpool = ctx.enter_context(tc.tile_pool(name="sqpool", bufs=2))
    pspool = ctx.enter_context(tc.tile_pool(name="pspool", bufs=2))
    xrpool = ctx.enter_context(tc.tile_pool(name="xrpool", bufs=2))
    xdpool = ctx.enter_context(tc.tile_pool(name="xdpool", bufs=2))
    smpool = ctx.enter_context(tc.tile_pool(name="smpool", bufs=2))
    tailpool = ctx.enter_context(tc.tile_pool(name="tailpool", bufs=2))

    # --- first chunk DMAs before constants, split in halves so the first
    # reduces can start as soon as the first 2 KiB/partition lands ---
    xsf_early = xs.rearrange("p d h w -> p (d h w)")
    xc_early = []
    for k in range(2):
        xc = xpool.tile([P, CHUNK], F32, tag="xc", name=f"xce{k}")
        half = CHUNK // 2
        for s in range(2):
            nc.sync.dma_start(
                out=xc[:, s * half:(s + 1) * half],
                in_=xsf_early[:, k * CHUNK + s * half:k * CHUNK +
                              (s + 1) * half])
        xc_early.append(xc)

    # --- constants ---
    ga_t = singles.tile([P, 32], F32)
    go_t = singles.tile([P, 32], F32)
    gw_t = singles.tile([P, 32], F32)
    bw_t = singles.tile([P, 32], F32)
    for r, t in enumerate((ga_t, go_t, gw_t, bw_t)):
        nc.sync.dma_start(out=t[:], in_=cons[r:r + 1, :].to_broadcast((P, 32)))
    ga16_t = singles.tile([P, 32], F16)
    nc.vector.tensor_scalar_mul(out=ga16_t[:], in0=ga_t[:], scalar1=1.0)
    go16_t = singles.tile([P, 32], F16)
    nc.vector.tensor_scalar_mul(out=go16_t[:], in0=go_t[:], scalar1=1.0)
    bw16_t = singles.tile([P, 32], F16)
    nc.vector.tensor_scalar_mul(out=bw16_t[:], in0=bw_t[:], scalar1=1.0)
    eps_t = singles.tile([P, 1], F32)
    nc.vector.memset(eps_t[:], EPS)

    xsf = xs.rearrange("p d h w -> p (d h w)")
    outf = out_ap.rearrange("p d h w -> p (d h w)")  # [P, 4096]

    # --- persistent staging ---
    # xh layout per pair: [P, 2 chunks, 16 h', 2 parity, 32 w'] fp16
    xh_pair = [singles.tile([P, 2, 16, 2, 32], F16, name=f"xh{i}")
               for i in range(4)]
    rstd_p = [singles.tile([P, 2 * ROWS], F32, name=f"rstd{i}")
              for i in range(4)]
    r1_p = [singles.tile([P, 2 * ROWS], F32, name=f"r1v{i}")
            for i in range(4)]
    r2_p = [singles.tile([P, 2 * ROWS], F32, name=f"r2v{i}")
            for i in range(4)]
    mr_half = [singles.tile([P, 4 * ROWS], F32, name=f"mr{i}")
               for i in range(2)]

    def dma_in(k):
        xc = xpool.tile([P, CHUNK], F32, tag="xc")
        nc.sync.dma_start(out=xc[:], in_=xsf[:, k * CHUNK:(k + 1) * CHUNK])
        return xc

    def stats(k, xc, half=None):
        """ACT square (parity-outer fp16) + DVE psq + row reduces.

        Reduces are SBUF-contention-immune, so they are what co-runs with
        GPSIMD ops; the psq TT is small.  half=0/1 processes only 32 rows
        (used to shorten the pipeline-fill on the first chunks)."""
        p, kk = k // 2, k % 2
        if half is not None:
            hr = ROWS // 2
            x4o = xc[:, half * CHUNK // 2:(half + 1) * CHUNK // 2].rearrange(
                "p (r v t) -> p t r v", v=32, t=2)
            sq4 = sqpool.tile([P, 2, hr, 32], F16, tag="sqh", bufs=1)
            nc.scalar.activation(sq4[:], x4o,
                                 mybir.ActivationFunctionType.Square)
            psq = pspool.tile([P, hr, 32], F16, tag="psqh", bufs=1)
            nc.vector.tensor_tensor(out=psq[:], in0=sq4[:, 0, :, :],
                                    in1=sq4[:, 1, :, :], op=ALU.add)
            lo = kk * ROWS + half * hr
            nc.vector.tensor_reduce(out=r2_p[p][:, lo:lo + hr], in_=psq[:],
                                    axis=mybir.AxisListType.X, op=ALU.add)
            x4 = xc[:, half * CHUNK // 2:(half + 1) * CHUNK // 2].rearrange(
                "p (r v t) -> p r t v", v=32, t=2)
            ps = pspool.tile([P, hr, 32], F32, tag="psh", bufs=1)
            nc.gpsimd.tensor_tensor(out=ps[:], in0=x4[:, :, 0, :],
                                    in1=x4[:, :, 1, :], op=ALU.add)
            nc.vector.tensor_reduce(out=r1_p[p][:, lo:lo + hr], in_=ps[:],
                                    axis=mybir.AxisListType.X, op=ALU.add)
            return
        # x viewed as [P, parity, row, w'] (parity OUTER -> contiguous halves)
        x4o = xc[:].rearrange("p (r v t) -> p t r v", v=32, t=2)
        sq4 = sqpool.tile([P, 2, ROWS, 32], F16, tag="sq")
        nc.scalar.activation(sq4[:], x4o,
                             mybir.ActivationFunctionType.Square)
        psq = pspool.tile([P, ROWS, 32], F16, tag="psq")
        nc.vector.tensor_tensor(out=psq[:], in0=sq4[:, 0, :, :],
                                in1=sq4[:, 1, :, :], op=ALU.add)
        nc.vector.tensor_reduce(out=r2_p[p][:, kk * ROWS:(kk + 1) * ROWS],
                                in_=psq[:], axis=mybir.AxisListType.X,
                                op=ALU.add)
        x4 = xc[:].rearrange("p (r v t) -> p r t v", v=32, t=2)
        ps = pspool.tile([P, ROWS, 32], F32, tag="ps")
        nc.gpsimd.tensor_tensor(out=ps[:], in0=x4[:, :, 0, :],
                                in1=x4[:, :, 1, :], op=ALU.add)
        nc.vector.tensor_reduce(out=r1_p[p][:, kk * ROWS:(kk + 1) * ROWS],
                                in_=ps[:], axis=mybir.AxisListType.X,
                                op=ALU.add)

    def smalls(p):
        """Stats recombination for a pair (128 rows): rstd, mr = 64*mu*rstd."""
        r1v, r2v = r1_p[p][:], r2_p[p][:]
        sqm = smpool.tile([P, 2 * ROWS], F32, tag="sqm")
        nc.vector.tensor_tensor(out=sqm[:], in0=r1v, in1=r1v, op=ALU.mult)
        # v64 = r2 - sqm/64  (= 64 * var)
        v64 = smpool.tile([P, 2 * ROWS], F32, tag="v64")
        nc.vector.scalar_tensor_tensor(out=v64[:], in0=sqm[:],
                                       scalar=-1.0 / W, in1=r2v,
                                       op0=ALU.mult, op1=ALU.add)
        sd = smpool.tile([P, 2 * ROWS], F32, tag="sd")
        nc.scalar.activation(sd[:], v64[:],
                             mybir.ActivationFunctionType.Sqrt,
                             bias=eps_t[:], scale=1.0 / W)
        rt = rstd_p[p]
        nc.vector.reciprocal(out=rt[:], in_=sd[:])
        mrh = mr_half[p // 2]
        nc.vector.tensor_tensor(out=mrh[:, (p % 2) * 128:(p % 2) * 128 + 128],
                                in0=r1v, in1=rt[:], op=ALU.mult)

    def xr_op(k, xc):
        """xr = x*rstd (fp16, deinterleaved out) on DVE (broadcast reads are
        full speed when GPSIMD is idle)."""
        p, kk = k // 2, k % 2
        rt = rstd_p[p][:, kk * ROWS:(kk + 1) * ROWS]  # [P, 64]
        x4 = xc[:].rearrange("p (r v t) -> p r t v", v=32, t=2)
        xr = xrpool.tile([P, ROWS, 2, 32], F16, tag="xr")
        rb = rt.unsqueeze(2).unsqueeze(3).to_broadcast((P, ROWS, 2, 32))
        nc.vector.tensor_tensor(out=xr[:], in0=x4, in1=rb, op=ALU.mult)
        return xr

    def pools(k, xr):
        """d-pool + h-pool into xh_half (DVE fp16 2x)."""
        # d-pool: [P, 2, 2048] -> [P, 2048] (contiguous halves)
        xd = xdpool.tile([P, CHUNK // 2], F16, tag="xd")
        xr2 = xr[:].rearrange("p r t v -> p (r t v)").rearrange(
            "p (s f) -> p s f", s=2)
        nc.vector.tensor_tensor(out=xd[:], in0=xr2[:, 0, :], in1=xr2[:, 1, :],
                                op=ALU.add)
        # h-pool: [P, 16, 2, 64] -> xh_half[:, k%4]; 64 = (t, v)
        xd3 = xd[:].rearrange("p (h s f) -> p h s f", s=2, f=64)
        xho = xh_pair[k // 2][:, k % 2, :, :, :].rearrange(
            "p h t v -> p h (t v)")
        nc.vector.tensor_tensor(out=xho, in0=xd3[:, :, 0, :],
                                in1=xd3[:, :, 1, :], op=ALU.add)

    def tail_dve(p):
        """Mean correction + gamma combine + beta + GELU for pair p."""
        mr5 = mr_half[p // 2][:, (p % 2) * 128:(p % 2) * 128 + 128].rearrange(
            "p (k d q t) -> p k d q t", k=2, d=2, t=2)
        mq1 = tailpool.tile([P, 2, 2, 16], F32, tag="mq1")
        nc.vector.tensor_tensor(out=mq1[:], in0=mr5[:, :, :, :, 0],
                                in1=mr5[:, :, :, :, 1], op=ALU.add)
        mq = tailpool.tile([P, 2, 16], F32, tag="mq")
        nc.vector.tensor_tensor(out=mq[:], in0=mq1[:, :, 0, :],
                                in1=mq1[:, :, 1, :], op=ALU.add)
        sh3 = (P, 32, 32)
        corr = tailpool.tile([P, 32, 32], F16, tag="corr")
        mqb = mq[:].rearrange("p k h -> p (k h)").unsqueeze(2).to_broadcast(
            sh3)
        nc.vector.tensor_tensor(out=corr[:], in0=mqb,
                                in1=_bcast(gw_t[:], sh3), op=ALU.mult)
        xh = xh_pair[p][:]  # [P, 2, 16, 2, 32]
        xhf = xh.rearrange("p k h t v -> p (k h) t v")
        t1 = tailpool.tile([P, 32, 32], F16, tag="t1")
        nc.vector.tensor_tensor(out=t1[:], in0=xhf[:, :, 0, :],
                                in1=_bcast(ga16_t[:], sh3), op=ALU.mult)
        t2 = tailpool.tile([P, 32, 32], F16, tag="t2")
        nc.vector.tensor_tensor(out=t2[:], in0=xhf[:, :, 1, :],
                                in1=_bcast(go16_t[:], sh3), op=ALU.mult)
        s_t = tailpool.tile([P, 32, 32], F16, tag="s")
        nc.vector.tensor_tensor(out=s_t[:], in0=t1[:], in1=t2[:], op=ALU.add)
        pre = tailpool.tile([P, 32, 32], F16, tag="pre")
        nc.vector.tensor_tensor(out=pre[:], in0=s_t[:], in1=corr[:],
                                op=ALU.subtract)
        pre2 = tailpool.tile([P, 32, 32], F16, tag="pre2")
        nc.vector.tensor_tensor(out=pre2[:], in0=pre[:],
                                in1=_bcast(bw16_t[:], sh3), op=ALU.add)
        res = tailpool.tile([P, 2 * 512], F32, tag="res")
        nc.scalar.activation(res[:], pre2[:].rearrange("p a b -> p (a b)"),
                             mybir.ActivationFunctionType.Gelu, scale=0.125)
        nc.sync.dma_start(out=outf[:, p * 1024:(p + 1) * 1024], in_=res[:])

    # ---- schedule: chunk cadence; GP xr(k) co-runs with DVE reduces of
    # stats(k+2); DVE TT pools run in the GP-idle tail of each slot ----
    xc_t = [None] * NCHUNK
    xc_t[0], xc_t[1] = xc_early
    for k in range(2, 4):
        xc_t[k] = dma_in(k)
    stats(0, xc_t[0], half=0)
    stats(0, xc_t[0], half=1)
    stats(1, xc_t[1], half=0)
    stats(1, xc_t[1], half=1)
    smalls(0)
    for k in range(NCHUNK):
        xr_k = xr_op(k, xc_t[k])
        if k + 2 < NCHUNK:
            if k + 4 < NCHUNK:
                xc_t[k + 4] = dma_in(k + 4)
            stats(k + 2, xc_t[k + 2])
            if (k + 2) % 2 == 1:
                smalls((k + 2) // 2)
        pools(k, xr_k)
        if k >= 1 and k % 2 == 1:
            tail_dve(k // 2)


_CACHE: dict = {}


def _get_compiled():
    if "nc" not in _CACHE:
        nc = bacc.Bacc("TRN2", target_bir_lowering=False, debug=False)
        xs = nc.dram_tensor("xs", [P, D, H, W], F32, kind="ExternalInput").ap()
        cons = nc.dram_tensor("cons", [4, 32], F32, kind="ExternalInput").ap()
        out = nc.dram_tensor(
            "out", [P, D // 2, H // 2, W // 2], F32, kind="ExternalOutput"
        ).ap()
        from contextlib import ExitStack

        with tile.TileContext(nc) as tc, ExitStack() as ctx:
            _kernel_body(ctx, tc, out, xs, cons)
        nc.compile()
        _CACHE["nc"] = nc
    return _CACHE["nc"]


def _make_cons(gamma: np.ndarray, beta: np.ndarray) -> np.ndarray:
    ga = gamma[0::2].astype(np.float64)
    go = gamma[1::2].astype(np.float64)
    # mr carries 64*mu*rstd -> fold the 1/64 into gw
    gw = (ga + go) / 64.0
    bw = 4.0 * (beta[0::2].astype(np.float64) + beta[1::2].astype(np.float64))
    return np.stack([ga, go, gw, bw]).astype(np.float32)


def kernel(x, sum_weight, gamma, beta, trace=False):
    del sum_weight  # cancels exactly in LayerNorm (shift invariance)
    nc = _get_compiled()
    x = np.ascontiguousarray(np.asarray(x), dtype=np.float32)
    cons = _make_cons(np.asarray(gamma), np.asarray(beta))
    in_maps = []
    for core in range(NCORES):
        shard = x[core * NPER:(core + 1) * NPER].reshape(P, D, H, W)
        in_maps.append({"xs": shard, "cons": cons})
    res = run_bass_kernel_spmd(nc, in_maps, core_ids=list(range(NCORES)),
                               trace=trace)
    out = np.concatenate(
        [
            res.results[i]["out"].reshape(NPER, C, D // 2, H // 2, W // 2)
            for i in range(NCORES)
        ],
        axis=0,
    )
    if trace:
        return out, res
    return out


if __name__ == "__main__":
    rng = np.random.default_rng(0)
    x = rng.standard_normal((N, C, D, H, W), dtype=np.float32)
    sw = rng.standard_normal((1,)).astype(np.float32)
    gamma = rng.random((W,), dtype=np.float32)
    beta = rng.standard_normal((W,)).astype(np.float32)
    y = kernel(x, sw, gamma, beta)
    print(y.shape, y.dtype)
